# revision 1
# baseline (speedup 1.0000x reference)
"""Causal self-attention (B=4, T=2048, C=2048, H=16) on 8 NeuronCores.

Sharding: core c = (b, g) with b = c // 2 (batch), g = c % 2 (head group of 8
heads = 1024 channels). Data parallel over B, tensor parallel over heads; the
output projection is computed per head-group and the two partials per batch
are summed on the host (+ bp).

Per-core device program (identical SPMD program, different data):
  phase 1: qT/kT = bf16 GEMMs (x bf16, W bf16, fp32 PSUM) + bias, kept
           RESIDENT in SBUF in [d, t] layout (no DRAM round-trip). v in
           natural [t, d] bf16 via DRAM (layout transpose). Loops are
           ordered so one stationary weight tile feeds 4 open PSUM
           accumulations.
  phase 2: per head, per 512-wide query chunk c: S^T[j, i] = kT_jb^T qT
           with keys on PARTITIONS. The additive attn mask folds into the
           exp as the scalar-engine per-partition bias; the causal mask is
           a DVE add on the 4 diagonal blocks only, restricted to the live
           column range. exp -> P^T bf16 (dead columns memset). Z row sums
           ride the PE: a 1-column ones stationary accumulates
           sum_j P^T[j, i] into a [1, 512] PSUM alongside the AV
           accumulation y^T = sum_jb V_jb^T P^T_jb. Z is broadcast back
           across partitions with a K=1 matmul and y^T is normalized with
           a DVE divide. No transposes, no reciprocals, no gpsimd.
  phase 3: out = yT^T Wp_g^T from SBUF-resident yT (bf16) and Wp (bf16),
           accumulating over the 8 head chunks with stationary reuse
           across 4 output column chunks -> DRAM fp32.
"""

import math

import numpy as np
import ml_dtypes

import concourse.bass as bass
import concourse.bacc as bacc
import concourse.mybir as mybir
from concourse.tile import TileContext
from concourse.bass_utils import run_bass_kernel_spmd

T = 2048
C = 2048
N_HEAD = 16
D = 128          # head dim
HG = 8           # heads per core
CG = HG * D      # 1024: per-core projection width
B = 4
N_CORES = 8
NEG = -1.0e30

F32 = mybir.dt.float32
F32R = mybir.dt.float32r
BF16 = mybir.dt.bfloat16

_NC_CACHE = None


def _build_program():
    nc = bacc.Bacc("TRN2", target_bir_lowering=False, debug=False)

    xT = nc.dram_tensor("xT", [C, T], BF16, kind="ExternalInput")
    wqT = nc.dram_tensor("wqT", [C, CG], BF16, kind="ExternalInput")
    wkT = nc.dram_tensor("wkT", [C, CG], BF16, kind="ExternalInput")
    wvT = nc.dram_tensor("wvT", [C, CG], BF16, kind="ExternalInput")
    bq = nc.dram_tensor("bq", [128, HG], F32, kind="ExternalInput")
    bk = nc.dram_tensor("bk", [128, HG], F32, kind="ExternalInput")
    bv = nc.dram_tensor("bv", [128, CG], F32, kind="ExternalInput")
    wpT = nc.dram_tensor("wpT", [CG, C], BF16, kind="ExternalInput")
    maskT = nc.dram_tensor("maskT", [128, 16], F32, kind="ExternalInput")
    cdg = nc.dram_tensor("cdg", [128, 4, 512], F32, kind="ExternalInput")
    onesr = nc.dram_tensor("onesr", [1, 128], F32R, kind="ExternalInput")
    onesc = nc.dram_tensor("onesc", [128, 1], BF16, kind="ExternalInput")
    out = nc.dram_tensor("out", [T, C], F32, kind="ExternalOutput")

    vd = nc.dram_tensor("vd", [T, CG], BF16)
    zd = nc.dram_tensor("zd", [HG * 4, 512], F32)
    rd = nc.dram_tensor("rd", [HG * 4, 512], F32R)

    add = mybir.AluOpType.add
    mult = mybir.AluOpType.mult
    Exp = mybir.ActivationFunctionType.Exp
    Copy = mybir.ActivationFunctionType.Copy

    with TileContext(nc) as tc:
        # ---- constants that live for the whole kernel ----
        with tc.tile_pool(name="const", bufs=1) as cpool:
            maskT_sb = cpool.tile([128, 16], F32)
            nc.sync.dma_start(out=maskT_sb, in_=maskT[:, :])
            cdg_sb = cpool.tile([128, 4, 512], F32)
            nc.sync.dma_start(out=cdg_sb, in_=cdg[:, :, :])
            ones_sb = cpool.tile([1, 128], F32R)
            nc.sync.dma_start(out=ones_sb, in_=onesr[:, :])
            onesc_sb = cpool.tile([128, 1], BF16)
            nc.sync.dma_start(out=onesc_sb, in_=onesc[:, :])

            # q/k stay resident in SBUF for the whole kernel
            with (
                tc.tile_pool(name="qk", bufs=1) as qkpool,
            ):
                qT_sb = qkpool.tile([128, HG, T], BF16)
                kT_sb = qkpool.tile([128, HG, T], BF16)

                # ================= phase 1: QKV projections =================
                with (
                    tc.tile_pool(name="p1x", bufs=1) as xpool,
                    tc.tile_pool(name="p1w", bufs=2) as wpool,
                    tc.tile_pool(name="p1b", bufs=1) as bpool,
                    tc.tile_pool(name="p1psqk", bufs=4, space="PSUM") as psqk1,
                    tc.tile_pool(name="p1psv", bufs=4, space="PSUM") as psv1,
                    tc.tile_pool(name="p1ov", bufs=4) as ovpool,
                ):
                    xt = xpool.tile([128, 16, T], BF16)
                    for cg in range(4):
                        nc.sync.dma_start(
                            out=xt[:, cg * 4:(cg + 1) * 4, :],
                            in_=xT[cg * 512:(cg + 1) * 512, :].rearrange(
                                "(cc p) t -> p cc t", p=128
                            ),
                        )
                    bq_sb = bpool.tile([128, HG], F32)
                    nc.sync.dma_start(out=bq_sb, in_=bq[:, :])
                    bk_sb = bpool.tile([128, HG], F32)
                    nc.sync.dma_start(out=bk_sb, in_=bk[:, :])
                    bv_sb = bpool.tile([128, CG], F32)
                    nc.sync.dma_start(out=bv_sb, in_=bv[:, :])

                    # q and k -> SBUF [d, t]; one weight tile feeds 4 open
                    # PSUM accumulations (cc outer, tr inner)
                    for w_dram, b_sb, o_sb in (
                        (wqT, bq_sb, qT_sb),
                        (wkT, bk_sb, kT_sb),
                    ):
                        for dc in range(HG):
                            wt = wpool.tile([128, 16, 128], BF16, tag="wqk")
                            nc.sync.dma_start(
                                out=wt,
                                in_=w_dram[:, dc * 128:(dc + 1) * 128].rearrange(
                                    "(cc p) d -> p cc d", p=128
                                ),
                            )
                            pss = [psqk1.tile([128, 512], F32, tag="ps1",
                                              name=f"ps1_{tr}")
                                   for tr in range(4)]
                            for cc in range(16):
                                for tr in range(4):
                                    nc.tensor.matmul(
                                        pss[tr],
                                        wt[:, cc, :],
                                        xt[:, cc, tr * 512:(tr + 1) * 512],
                                        start=(cc == 0),
                                        stop=(cc == 15),
                                    )
                            for tr in range(4):
                                nc.vector.tensor_scalar_add(
                                    o_sb[:, dc, tr * 512:(tr + 1) * 512],
                                    pss[tr], b_sb[:, dc:dc + 1]
                                )

                    # v: natural layout [t, d], bf16, via DRAM
                    for dr in range(4):
                        wv_t = wpool.tile([128, 16, 256], BF16, tag="wv")
                        nc.sync.dma_start(
                            out=wv_t,
                            in_=wvT[:, dr * 256:(dr + 1) * 256].rearrange(
                                "(cc p) d -> p cc d", p=128
                            ),
                        )
                        for tcb in range(16):
                            ps = psv1.tile([128, 256], F32, tag="psv")
                            for cc in range(16):
                                nc.tensor.matmul(
                                    ps,
                                    xt[:, cc, tcb * 128:(tcb + 1) * 128],
                                    wv_t[:, cc, :],
                                    start=(cc == 0),
                                    stop=(cc == 15),
                                )
                            vb = ovpool.tile([128, 256], BF16, tag="ov")
                            nc.vector.tensor_tensor(
                                vb, ps, bv_sb[:, dr * 256:(dr + 1) * 256], add
                            )
                            nc.sync.dma_start(
                                out=vd[tcb * 128:(tcb + 1) * 128,
                                       dr * 256:(dr + 1) * 256],
                                in_=vb,
                            )

                # persistent across phase 2 -> 3: y^T and the proj weights
                with (
                    tc.tile_pool(name="yt", bufs=1) as ytpool,
                    tc.tile_pool(name="wp", bufs=1) as wppool,
                ):
                    yT_sb = ytpool.tile([128, HG, T], BF16)
                    wp_sb = wppool.tile([128, HG, C], BF16)
                    nc.sync.dma_start(
                        out=wp_sb,
                        in_=wpT.rearrange("(h p) c -> p h c", p=128),
                    )

                    # ============= phase 2: attention per head =============
                    with (
                        tc.tile_pool(name="p2v", bufs=2) as vpool,
                        tc.tile_pool(name="p2pt", bufs=2) as ptpool,
                        tc.tile_pool(name="p2z", bufs=2) as zpool,
                        tc.tile_pool(name="p2ps", bufs=2, space="PSUM") as psst,
                        tc.tile_pool(name="p2psy", bufs=3, space="PSUM") as psy,
                        tc.tile_pool(name="p2psz", bufs=2, space="PSUM") as psz,
                        tc.tile_pool(name="p2psb", bufs=1, space="PSUM") as psb,
                    ):
                        for h in range(HG):
                            vh = vpool.tile([128, 16, 128], BF16, tag="vh")
                            nc.sync.dma_start(
                                out=vh,
                                in_=vd[:, h * 128:(h + 1) * 128].rearrange(
                                    "(tc p) d -> p tc d", p=128
                                ),
                            )
                            # software pipeline: S-stage(c), AV-stage(c-1)
                            pts = [None] * 4
                            for c in range(5):
                                if c < 4:
                                    njb = 4 * (c + 1)
                                    pt = ptpool.tile([128, 16, 512], BF16,
                                                     tag="pt")
                                    pts[c] = pt
                                    for jb in range(njb):
                                        s = jb - 4 * c  # >=0 on diagonal
                                        lo = s * 128 if s > 0 else 0
                                        w = 512 - lo
                                        ps = psst.tile([128, 512], F32,
                                                       tag="ps")
                                        nc.tensor.matmul(
                                            ps[:, lo:512],
                                            kT_sb[:, h,
                                                  jb * 128:(jb + 1) * 128],
                                            qT_sb[:, h,
                                                  c * 512 + lo:(c + 1) * 512],
                                            start=True,
                                            stop=True,
                                        )
                                        if s >= 0:
                                            nc.vector.tensor_tensor(
                                                ps[:, lo:512], ps[:, lo:512],
                                                cdg_sb[:, s, lo:512], add,
                                            )
                                        if lo > 0:
                                            nc.vector.memset(
                                                pt[:, jb, 0:lo], 0.0
                                            )
                                        nc.scalar.activation(
                                            pt[:, jb, lo:512], ps[:, lo:512],
                                            Exp,
                                            bias=maskT_sb[:, jb:jb + 1],
                                        )
                                if c > 0:
                                    cc_ = c - 1
                                    njb = 4 * (cc_ + 1)
                                    pt = pts[cc_]
                                    yps = psy.tile([128, 512], F32, tag="yps")
                                    zps = psz.tile([1, 512], F32, tag="zps")
                                    for jb in range(njb):
                                        nc.tensor.matmul(
                                            yps,
                                            vh[:, jb, :],
                                            pt[:, jb, :],
                                            start=(jb == 0),
                                            stop=(jb == njb - 1),
                                        )
                                    za = zpool.tile([128, 512], BF16,
                                                    tag="za", name="za")
                                    zb = zpool.tile([128, 512], BF16,
                                                    tag="zb", name="zb")
                                    nc.vector.tensor_copy(za, pt[:, 0, :])
                                    nc.vector.tensor_copy(zb, pt[:, 1, :])
                                    for jb in range(2, njb, 2):
                                        nc.vector.tensor_tensor(
                                            za, za, pt[:, jb, :], add)
                                    for jb in range(3, njb, 2):
                                        nc.vector.tensor_tensor(
                                            zb, zb, pt[:, jb, :], add)
                                    nc.tensor.matmul(zps, onesc_sb, za,
                                                     start=True, stop=False)
                                    nc.tensor.matmul(zps, onesc_sb, zb,
                                                     start=False, stop=True)
                                    # 1/Z: bounce Z through DRAM to put i on
                                    # partitions (fast 128-lane reciprocal),
                                    # bounce back to a row, broadcast via a
                                    # K=1 matmul, multiply.
                                    hc = h * 4 + cc_
                                    zsb = zpool.tile([1, 512], F32, tag="zsb")
                                    nc.vector.tensor_copy(zsb, zps)
                                    nc.sync.dma_start(
                                        out=zd[hc:hc + 1, :], in_=zsb
                                    )
                                    zT = zpool.tile([128, 4], F32, tag="zT")
                                    nc.sync.dma_start(
                                        out=zT,
                                        in_=zd[hc:hc + 1, :].rearrange(
                                            "p (a b) -> (p b) a", a=4, b=128
                                        ),
                                    )
                                    rT = zpool.tile([128, 4], F32R, tag="rT")
                                    with nc.allow_low_precision(
                                        reason="f32r is f32 bits; matmul rhs"
                                    ):
                                        nc.vector.reciprocal(rT, zT)
                                    nc.sync.dma_start(
                                        out=rd[hc:hc + 1, :].rearrange(
                                            "p (a b) -> (p b) a", a=4, b=128
                                        ),
                                        in_=rT,
                                    )
                                    rrow = zpool.tile([1, 512], F32R,
                                                      tag="rrow")
                                    nc.sync.dma_start(
                                        out=rrow, in_=rd[hc:hc + 1, :]
                                    )
                                    rbc = psb.tile([128, 512], F32, tag="rbc")
                                    nc.tensor.matmul(
                                        rbc, ones_sb, rrow,
                                        start=True, stop=True,
                                    )
                                    rbs = zpool.tile([128, 512], F32,
                                                     tag="rbs")
                                    nc.vector.tensor_copy(rbs, rbc)
                                    nc.vector.tensor_tensor(
                                        yT_sb[:, h,
                                              cc_ * 512:(cc_ + 1) * 512],
                                        yps, rbs, mult,
                                    )

                    # ============= phase 3: output projection =============
                    with (
                        tc.tile_pool(name="p3ps", bufs=4, space="PSUM") as ps3,
                        tc.tile_pool(name="p3o", bufs=4) as op3,
                    ):
                        for tcb in range(16):
                            pss = [ps3.tile([128, 512], F32, tag="ps3",
                                            name=f"ps3_{cr}")
                                   for cr in range(4)]
                            for h in range(HG):
                                for cr in range(4):
                                    nc.tensor.matmul(
                                        pss[cr],
                                        yT_sb[:, h, tcb * 128:(tcb + 1) * 128],
                                        wp_sb[:, h, cr * 512:(cr + 1) * 512],
                                        start=(h == 0),
                                        stop=(h == HG - 1),
                                    )
                            for cr in range(4):
                                ob = op3.tile([128, 512], F32, tag="ob")
                                nc.scalar.activation(ob, pss[cr], Copy)
                                nc.sync.dma_start(
                                    out=out[tcb * 128:(tcb + 1) * 128,
                                            cr * 512:(cr + 1) * 512],
                                    in_=ob,
                                )
    nc.compile()
    return nc


def get_nc():
    global _NC_CACHE
    if _NC_CACHE is None:
        _NC_CACHE = _build_program()
    return _NC_CACHE


def prep_core_inputs(inputs):
    """Host-side sharding / layout prep: slice per (b, g), transpose to the
    layouts the device program wants, fold the 1/sqrt(d) softmax scale into
    Wq/bq."""
    f = lambda a: np.asarray(a, dtype=np.float32)
    bf = ml_dtypes.bfloat16
    x = f(inputs["x"])
    am = f(inputs["attn_mask"])
    Wq, bq_ = f(inputs["Wq"]), f(inputs["bq"])
    Wk, bk_ = f(inputs["Wk"]), f(inputs["bk"])
    Wv, bv_ = f(inputs["Wv"]), f(inputs["bv"])
    Wp = f(inputs["Wp"])
    scale = 1.0 / math.sqrt(D)

    # causal tiles in S^T layout: for diagonal block s (0..3) of a 512-wide
    # query chunk, partition p = key offset within the 128-block, column
    # i_local in [0, 512): masked (i < j) iff i_local < s*128 + p.
    ii = np.arange(512)[None, :]
    pp = np.arange(128)[:, None]
    cdg_t = np.stack(
        [np.where(ii < s * 128 + pp, NEG, 0.0) for s in range(4)], axis=1
    ).astype(np.float32)  # [128, 4, 512]

    per_g = []
    for g in range(2):
        sl = slice(g * CG, (g + 1) * CG)
        per_g.append(dict(
            wqT=(np.ascontiguousarray(Wq[sl].T) * scale).astype(bf),
            wkT=np.ascontiguousarray(Wk[sl].T).astype(bf),
            wvT=np.ascontiguousarray(Wv[sl].T).astype(bf),
            bq=np.ascontiguousarray((bq_[sl] * scale).reshape(HG, 128).T),
            bk=np.ascontiguousarray(bk_[sl].reshape(HG, 128).T),
            bv=np.ascontiguousarray(np.broadcast_to(bv_[sl], (128, CG))),
            wpT=np.ascontiguousarray(Wp[:, sl].T).astype(bf),
        ))

    onesr_t = np.ones((1, 128), dtype=np.float32)
    onesc_t = np.ones((128, 1), dtype=bf)

    in_maps = []
    for core in range(N_CORES):
        b, g = core // 2, core % 2
        m = dict(per_g[g])
        m["xT"] = np.ascontiguousarray(x[b].T).astype(bf)
        m["maskT"] = np.ascontiguousarray(
            am[b, 0, 0, :].reshape(16, 128).T
        )
        m["cdg"] = cdg_t
        m["onesr"] = onesr_t
        m["onesc"] = onesc_t
        in_maps.append(m)
    return in_maps


def run(inputs, trace=False):
    nc = get_nc()
    in_maps = prep_core_inputs(inputs)
    rr = run_bass_kernel_spmd(nc, in_maps, list(range(N_CORES)), trace=trace)
    bp = np.asarray(inputs["bp"], dtype=np.float32)
    y = np.empty((B, T, C), dtype=np.float32)
    for b in range(B):
        y[b] = rr.results[2 * b]["out"] + rr.results[2 * b + 1]["out"] + bp[None, :]
    return y, rr


def kernel(**inputs):
    y, _ = run(inputs)
    return y



# revision 11
# speedup vs baseline: 1.5325x; 1.5325x over previous
"""Causal self-attention (B=4, T=2048, C=2048, H=16) on 8 NeuronCores.

Sharding: core c = (b, g) with b = c // 2 (batch), g = c % 2 (head group of 8
heads = 1024 channels). Data parallel over B, tensor parallel over heads; the
output projection is computed per head-group and the two partials per batch
are summed on the host (+ bp).

Device program: a fused per-head pipeline. The QKV projections for head h+1
(pure GEMM) are interleaved into head h's attention steps so the scalar
(exp) and vector (sums/normalize) work hides under tensor-engine GEMMs and
the PE never idles long enough to re-throttle (HAM).

Per chunk step s = (h, c) with njb = 4(c+1) key blocks:
  PE:  S^T blocks (keys on partitions)  ->  AV accumulation of chunk s-1
       -> Z broadcast matmuls (ones128^T @ za/zb) -> projection GEMM slice
       (q/k for head h+1, v for head-pair h//2+1)
  ACT: exp of each S block with the additive attn mask as per-partition bias
  DVE: 1/Z (128-wide, in SBUF), yT normalize mult of chunk s-1, causal
       staircase zeroing (one bf16 0/1 multiply per chunk), the za/zb
       pairwise partial-sum tree, projection bias adds.
v stays resident in SBUF (no DRAM round trip); Z broadcast comes straight
from a [128,128] ones stationary so there is no transpose/DRAM bounce on the
PE critical path. Phase 3 (out = yT^T Wp) streams Wp after the slots finish.
"""

import math

import numpy as np
import ml_dtypes

import concourse.bass as bass
import concourse.bacc as bacc
import concourse.mybir as mybir
from concourse.tile import TileContext
from concourse.bass_utils import run_bass_kernel_spmd

T = 2048
C = 2048
N_HEAD = 16
D = 128          # head dim
HG = 8           # heads per core
CG = HG * D      # 1024: per-core projection width
B = 4
N_CORES = 8

F32 = mybir.dt.float32
BF16 = mybir.dt.bfloat16

_NC_CACHE = None


def _build_program():
    nc = bacc.Bacc("TRN2", target_bir_lowering=False, debug=False)

    xT = nc.dram_tensor("xT", [C, T], BF16, kind="ExternalInput")
    wqT = nc.dram_tensor("wqT", [C, CG], BF16, kind="ExternalInput")
    wkT = nc.dram_tensor("wkT", [C, CG], BF16, kind="ExternalInput")
    wvT = nc.dram_tensor("wvT", [C, CG], BF16, kind="ExternalInput")
    bq = nc.dram_tensor("bq", [128, HG], F32, kind="ExternalInput")
    bk = nc.dram_tensor("bk", [128, HG], F32, kind="ExternalInput")
    bvb = nc.dram_tensor("bvb", [128, CG], BF16, kind="ExternalInput")
    wpT = nc.dram_tensor("wpT", [CG, C], BF16, kind="ExternalInput")
    maskT = nc.dram_tensor("maskT", [128, 16], F32, kind="ExternalInput")
    cdg01 = nc.dram_tensor("cdg01", [128, 4, 512], BF16, kind="ExternalInput")
    ones128 = nc.dram_tensor("ones128", [128, 128], BF16, kind="ExternalInput")
    out = nc.dram_tensor("out", [T, C], F32, kind="ExternalOutput")

    add = mybir.AluOpType.add
    mult = mybir.AluOpType.mult
    Exp = mybir.ActivationFunctionType.Exp
    Copy = mybir.ActivationFunctionType.Copy

    with TileContext(nc) as tc:
        # ---- constants that live for the whole kernel ----
        with tc.tile_pool(name="const", bufs=1) as cpool:
            maskT_sb = cpool.tile([128, 16], F32)
            nc.scalar.dma_start(out=maskT_sb, in_=maskT[:, :])
            cdg01_sb = cpool.tile([128, 4, 512], BF16)
            nc.scalar.dma_start(out=cdg01_sb, in_=cdg01[:, :, :])
            ones_sb = cpool.tile([128, 128], BF16)
            nc.scalar.dma_start(out=ones_sb, in_=ones128[:, :])
            bq_sb = cpool.tile([128, HG], F32)
            nc.scalar.dma_start(out=bq_sb, in_=bq[:, :])
            bk_sb = cpool.tile([128, HG], F32)
            nc.scalar.dma_start(out=bk_sb, in_=bk[:, :])
            bv_sb = cpool.tile([128, CG], BF16)
            nc.scalar.dma_start(out=bv_sb, in_=bvb[:, :])

            with tc.tile_pool(name="yt", bufs=1) as ytpool:
                yT_sb = ytpool.tile([128, HG, T], BF16)

                with (
                    tc.tile_pool(name="xx", bufs=1) as xpool,
                    tc.tile_pool(name="qk", bufs=2) as qkpool,
                    tc.tile_pool(name="vh", bufs=2) as vhpool,
                    tc.tile_pool(name="wv", bufs=2) as wvpool,
                    tc.tile_pool(name="wqk", bufs=2) as wqkpool,
                    tc.tile_pool(name="pt", bufs=2) as ptpool,
                    tc.tile_pool(name="zz", bufs=2) as zpool,
                    tc.tile_pool(name="psqk", bufs=2, space="PSUM") as psqk,
                    tc.tile_pool(name="psv", bufs=2, space="PSUM") as psv,
                    tc.tile_pool(name="psst", bufs=3, space="PSUM") as psst,
                    tc.tile_pool(name="psy", bufs=1, space="PSUM") as psy,
                ):
                    # ---------- startup DMAs across 4 queues ----------
                    xt = xpool.tile([128, 16, T], BF16)
                    xq = [nc.sync, nc.scalar, nc.gpsimd, nc.sync]
                    for cg in range(4):
                        xq[cg].dma_start(
                            out=xt[:, cg * 4:(cg + 1) * 4, :],
                            in_=xT[cg * 512:(cg + 1) * 512, :].rearrange(
                                "(cc p) t -> p cc t", p=128
                            ),
                        )

                    wq_t = {}   # (head, 'q'/'k') -> weight tile
                    wv_t = {}   # pair -> weight tile
                    q_ring = {}
                    k_ring = {}
                    vh_ring = {}

                    def dma_wqk(dc, which, queue):
                        w_dram = wqT if which == "q" else wkT
                        wt = wqkpool.tile([128, 16, 128], BF16,
                                          tag=f"w{which}",
                                          name=f"w{which}{dc}")
                        queue.dma_start(
                            out=wt,
                            in_=w_dram[:, dc * 128:(dc + 1) * 128].rearrange(
                                "(cc p) d -> p cc d", p=128
                            ),
                        )
                        wq_t[(dc, which)] = wt

                    def dma_wv(p, queue):
                        wt = wvpool.tile([128, 16, 256], BF16, tag="wv",
                                         name=f"wv{p}")
                        queue.dma_start(
                            out=wt,
                            in_=wvT[:, p * 256:(p + 1) * 256].rearrange(
                                "(cc p) d -> p cc d", p=128
                            ),
                        )
                        wv_t[p] = wt

                    def qk_piece(dc, which, tr_half):
                        """Thunks for 32 matmuls + 2 bias drains: half of q
                        or k for head dc."""
                        ring = q_ring if which == "q" else k_ring
                        if dc not in ring:
                            ring[dc] = qkpool.tile(
                                [128, T], BF16, tag=f"{which}ring",
                                name=f"{which}{dc}",
                            )
                        wt = wq_t[(dc, which)]
                        b_sb = bq_sb if which == "q" else bk_sb
                        pss = [psqk.tile([128, 512], F32, tag="qkps",
                                         name=f"qkps{t2}") for t2 in range(2)]
                        thunks = []

                        def mm(cc, t2):
                            tr = 2 * tr_half + t2
                            nc.tensor.matmul(
                                pss[t2],
                                wt[:, cc, :],
                                xt[:, cc, tr * 512:(tr + 1) * 512],
                                start=(cc == 0),
                                stop=(cc == 15),
                            )

                        def drain(t2):
                            tr = 2 * tr_half + t2
                            nc.vector.tensor_scalar_add(
                                ring[dc][:, tr * 512:(tr + 1) * 512],
                                pss[t2], b_sb[:, dc:dc + 1],
                            )

                        for cc in range(16):
                            for t2 in range(2):
                                thunks.append(
                                    lambda cc=cc, t2=t2: mm(cc, t2))
                        for t2 in range(2):
                            thunks.append(lambda t2=t2: drain(t2))
                        return thunks

                    def v_group(p, tcb):
                        """Thunks for 16 matmuls (N=256) + bias drain: one
                        t-block of v for head pair p."""
                        if p not in vh_ring:
                            vh_ring[p] = vhpool.tile(
                                [128, 16, 256], BF16, tag="vh", name=f"vh{p}",
                            )
                        ps = psv.tile([128, 256], F32, tag="vps", name="vps")

                        def mm(cc):
                            nc.tensor.matmul(
                                ps,
                                xt[:, cc, tcb * 128:(tcb + 1) * 128],
                                wv_t[p][:, cc, :],
                                start=(cc == 0),
                                stop=(cc == 15),
                            )

                        def drain():
                            nc.vector.tensor_tensor(
                                vh_ring[p][:, tcb, :], ps,
                                bv_sb[:, p * 256:(p + 1) * 256], add,
                            )

                        return [lambda cc=cc: mm(cc) for cc in range(16)] + \
                            [drain]

                    # ---------- prologue ----------
                    dma_wqk(0, "q", nc.sync)
                    dma_wqk(0, "k", nc.scalar)
                    dma_wv(0, nc.scalar)
                    dma_wv(1, nc.gpsimd)
                    dma_wqk(1, "q", nc.sync)
                    dma_wqk(1, "k", nc.gpsimd)

                    for th in range(2):
                        for t in qk_piece(0, "q", th):
                            t()
                    for th in range(2):
                        for t in qk_piece(0, "k", th):
                            t()
                    for tcb in range(16):
                        for t in v_group(0, tcb):
                            t()

                    # ---------- fused head/chunk steps ----------
                    # per-chunk state kept across steps for the s-1 tail
                    state = {}

                    def av_thunks(h, c, pt, njb, za, zb):
                        """Thunks: AV accumulation, Z broadcast matmuls, then
                        1/Z + yT normalize (DVE) for chunk (h, c)."""
                        vh = vh_ring[h // 2]
                        dlo = (h % 2) * 128
                        yps = psy.tile([128, 512], F32, tag="y", name="y")
                        zbc = psv.tile([128, 512], F32, tag="vps", name="zbc")

                        def av_mm(jb):
                            nc.tensor.matmul(
                                yps,
                                vh[:, jb, dlo:dlo + 128],
                                pt[:, jb, :],
                                start=(jb == 0),
                                stop=(jb == njb - 1),
                            )

                        def zm_a():
                            nc.tensor.matmul(zbc, ones_sb, za, start=True,
                                             stop=False)

                        def zm_b_norm():
                            nc.tensor.matmul(zbc, ones_sb, zb, start=False,
                                             stop=True)
                            rr = zpool.tile([128, 512], F32, tag="rr",
                                            name="rr")
                            with nc.allow_low_precision(
                                reason="z>0, fp32 reciprocal on DVE"
                            ):
                                nc.vector.reciprocal(rr, zbc)
                            nc.vector.tensor_tensor(
                                yT_sb[:, h, c * 512:(c + 1) * 512], yps, rr,
                                mult,
                            )

                        return [lambda jb=jb: av_mm(jb)
                                for jb in range(njb)] + [zm_a, zm_b_norm]

                    for s in range(33):
                        # ---- build this step's filler (prev chunk tail +
                        # projection GEMMs) ----
                        filler = []
                        if s >= 1:
                            hp, cp = divmod(s - 1, 4)
                            ptp, njbp, zap, zbp = state.pop((hp, cp))
                            filler += av_thunks(hp, cp, ptp, njbp, zap, zbp)
                        if s < 32:
                            h, c = divmod(s, 4)
                            # weight prefetch for upcoming work
                            if c == 0 and h + 2 < HG:
                                dma_wqk(h + 2, "q", nc.sync)
                                dma_wqk(h + 2, "k", nc.gpsimd)
                            if c == 2 and h % 2 == 1:
                                p = (h + 3) // 2
                                if p <= 3:
                                    dma_wv(p, nc.gpsimd)
                            if h + 1 < HG:
                                which = "q" if c < 2 else "k"
                                filler += qk_piece(h + 1, which, c % 2)
                            p = h // 2 + 1
                            if p <= 3:
                                tb = (h % 2) * 8 + c * 2
                                filler += v_group(p, tb)
                                filler += v_group(p, tb + 1)

                        fi = iter(filler)

                        def pull(n):
                            for _ in range(n):
                                t = next(fi, None)
                                if t is None:
                                    return
                                t()

                        # ---- S blocks + exp, interleaved with filler ----
                        if s < 32:
                            njb = 4 * (c + 1)
                            pt = ptpool.tile([128, 16, 512], BF16, tag="pt")
                            # clear the stale [0, lo) regions of the diagonal
                            # blocks (read by the staircase multiply / AV)
                            for sdg in range(1, 4):
                                nc.vector.memset(
                                    pt[:, 4 * c + sdg, 0:sdg * 128], 0.0)
                            for jb in range(njb):
                                sdg = jb - 4 * c
                                lo = sdg * 128 if sdg > 0 else 0
                                ps = psst.tile([128, 512], F32, tag="s",
                                               name="s")
                                nc.tensor.matmul(
                                    ps[:, lo:512],
                                    k_ring[h][:, jb * 128:(jb + 1) * 128],
                                    q_ring[h][:, c * 512 + lo:(c + 1) * 512],
                                    start=True,
                                    stop=True,
                                )
                                nc.scalar.activation(
                                    pt[:, jb, lo:512], ps[:, lo:512], Exp,
                                    bias=maskT_sb[:, jb:jb + 1],
                                )
                                pull(3)
                            # staircase zero of the diagonal blocks
                            nc.vector.tensor_tensor(
                                pt[:, 4 * c:4 * c + 4, :],
                                pt[:, 4 * c:4 * c + 4, :],
                                cdg01_sb[:, :, :], mult,
                            )
                            # pairwise partial-sum tree -> za, zb (bf16)
                            za = zpool.tile([128, 512], BF16, tag="za",
                                            name="za")
                            zb = zpool.tile([128, 512], BF16, tag="zb",
                                            name="zb")
                            nc.vector.tensor_tensor(za, pt[:, 0, :],
                                                    pt[:, 2, :], add)
                            nc.vector.tensor_tensor(zb, pt[:, 1, :],
                                                    pt[:, 3, :], add)
                            for base in range(4, njb, 2):
                                nc.vector.tensor_tensor(
                                    za, za, pt[:, base, :], add)
                                nc.vector.tensor_tensor(
                                    zb, zb, pt[:, base + 1, :], add)
                            state[(h, c)] = (pt, njb, za, zb)
                        # ---- flush remaining filler ----
                        pull(len(filler))

                # ---------- phase 3: out = yT^T @ WpT ----------
                with (
                    tc.tile_pool(name="wp", bufs=1) as wppool,
                    tc.tile_pool(name="p3ps", bufs=4, space="PSUM") as ps3,
                    tc.tile_pool(name="p3o", bufs=4) as op3,
                ):
                    wp_sb = wppool.tile([128, HG, C], BF16)
                    oq = [nc.sync, nc.scalar, nc.gpsimd]
                    for hh in range(HG):
                        oq[hh % 3].dma_start(
                            out=wp_sb[:, hh, :],
                            in_=wpT[hh * 128:(hh + 1) * 128, :],
                        )
                    for tcb in range(16):
                        pss = [ps3.tile([128, 512], F32, tag="ps3",
                                        name=f"ps3_{cr}")
                               for cr in range(4)]
                        for hh in range(HG):
                            for cr in range(4):
                                nc.tensor.matmul(
                                    pss[cr],
                                    yT_sb[:, hh, tcb * 128:(tcb + 1) * 128],
                                    wp_sb[:, hh, cr * 512:(cr + 1) * 512],
                                    start=(hh == 0),
                                    stop=(hh == HG - 1),
                                )
                        for cr in range(4):
                            ob = op3.tile([128, 512], F32, tag="ob")
                            nc.scalar.activation(ob, pss[cr], Copy)
                            nc.sync.dma_start(
                                out=out[tcb * 128:(tcb + 1) * 128,
                                        cr * 512:(cr + 1) * 512],
                                in_=ob,
                            )
    nc.compile()
    return nc


def get_nc():
    global _NC_CACHE
    if _NC_CACHE is None:
        _NC_CACHE = _build_program()
    return _NC_CACHE


def prep_core_inputs(inputs):
    """Host-side sharding / layout prep: slice per (b, g), transpose to the
    layouts the device program wants, fold the 1/sqrt(d) softmax scale into
    Wq/bq."""
    f = lambda a: np.asarray(a, dtype=np.float32)
    bf = ml_dtypes.bfloat16
    x = f(inputs["x"])
    am = f(inputs["attn_mask"])
    Wq, bq_ = f(inputs["Wq"]), f(inputs["bq"])
    Wk, bk_ = f(inputs["Wk"]), f(inputs["bk"])
    Wv, bv_ = f(inputs["Wv"]), f(inputs["bv"])
    Wp = f(inputs["Wp"])
    scale = 1.0 / math.sqrt(D)

    # 0/1 staircase in S^T layout: for diagonal block s (0..3) of a 512-wide
    # query chunk, partition p = key offset within the 128-block, column
    # i_local in [0, 512): masked (dead) iff i_local < s*128 + p.
    ii = np.arange(512)[None, :]
    pp = np.arange(128)[:, None]
    cdg01_t = np.stack(
        [np.where(ii < s * 128 + pp, 0.0, 1.0) for s in range(4)], axis=1
    ).astype(bf)  # [128, 4, 512]

    per_g = []
    for g in range(2):
        sl = slice(g * CG, (g + 1) * CG)
        per_g.append(dict(
            wqT=(np.ascontiguousarray(Wq[sl].T) * scale).astype(bf),
            wkT=np.ascontiguousarray(Wk[sl].T).astype(bf),
            wvT=np.ascontiguousarray(Wv[sl].T).astype(bf),
            bq=np.ascontiguousarray((bq_[sl] * scale).reshape(HG, 128).T),
            bk=np.ascontiguousarray(bk_[sl].reshape(HG, 128).T),
            bvb=np.ascontiguousarray(
                np.broadcast_to(bv_[sl], (128, CG))
            ).astype(bf),
            wpT=np.ascontiguousarray(Wp[:, sl].T).astype(bf),
        ))

    ones_t = np.ones((128, 128), dtype=bf)

    in_maps = []
    for core in range(N_CORES):
        b, g = core // 2, core % 2
        m = dict(per_g[g])
        m["xT"] = np.ascontiguousarray(x[b].T).astype(bf)
        m["maskT"] = np.ascontiguousarray(
            am[b, 0, 0, :].reshape(16, 128).T
        )
        m["cdg01"] = cdg01_t
        m["ones128"] = ones_t
        in_maps.append(m)
    return in_maps


def run(inputs, trace=False):
    nc = get_nc()
    in_maps = prep_core_inputs(inputs)
    rr = run_bass_kernel_spmd(nc, in_maps, list(range(N_CORES)), trace=trace)
    bp = np.asarray(inputs["bp"], dtype=np.float32)
    y = np.empty((B, T, C), dtype=np.float32)
    for b in range(B):
        y[b] = rr.results[2 * b]["out"] + rr.results[2 * b + 1]["out"] + bp[None, :]
    return y, rr


def kernel(**inputs):
    y, _ = run(inputs)
    return y


# revision 18
# speedup vs baseline: 1.5936x; 1.0399x over previous
"""Causal self-attention (B=4, T=2048, C=2048, H=16) on 8 NeuronCores.

Sharding: core c = (b, g) with b = c // 2 (batch), g = c % 2 (head group of 8
heads = 1024 channels). Data parallel over B, tensor parallel over heads; the
output projection is computed per head-group and the two partials per batch
are summed on the host (+ bp).

Device program: a fused per-head pipeline. The QKV projections for head h+1
(pure GEMM) are interleaved into head h's attention steps so the scalar
(exp) and vector (sums/normalize) work hides under tensor-engine GEMMs and
the PE never idles long enough to re-throttle (HAM).

Per chunk step s = (h, c) with njb = 4(c+1) key blocks:
  PE:  S^T blocks (keys on partitions)  ->  AV accumulation of chunk s-1
       -> Z broadcast matmuls (ones128^T @ za/zb) -> projection GEMM slice
       (q/k for head h+1, v for head-pair h//2+1)
  ACT: exp of each S block with the additive attn mask as per-partition bias
  DVE: 1/Z (128-wide, in SBUF), yT normalize mult of chunk s-1, causal
       staircase zeroing (one bf16 0/1 multiply per chunk), the za/zb
       pairwise partial-sum tree, projection bias adds.
v stays resident in SBUF (no DRAM round trip); Z broadcast comes straight
from a [128,128] ones stationary so there is no transpose/DRAM bounce on the
PE critical path. Phase 3 (out = yT^T Wp) streams Wp after the slots finish.
"""

import math

import numpy as np
import ml_dtypes

import concourse.bass as bass
import concourse.bacc as bacc
import concourse.mybir as mybir
from concourse.tile import TileContext
from concourse.bass_utils import run_bass_kernel_spmd

T = 2048
C = 2048
N_HEAD = 16
D = 128          # head dim
HG = 8           # heads per core
CG = HG * D      # 1024: per-core projection width
B = 4
N_CORES = 8

F32 = mybir.dt.float32
BF16 = mybir.dt.bfloat16

_NC_CACHE = None


def _build_program():
    nc = bacc.Bacc("TRN2", target_bir_lowering=False, debug=False)

    xT = nc.dram_tensor("xT", [C, T], BF16, kind="ExternalInput")
    wqT = nc.dram_tensor("wqT", [C, CG], BF16, kind="ExternalInput")
    wkT = nc.dram_tensor("wkT", [C, CG], BF16, kind="ExternalInput")
    wvT = nc.dram_tensor("wvT", [C, CG], BF16, kind="ExternalInput")
    bq = nc.dram_tensor("bq", [128, HG], F32, kind="ExternalInput")
    bk = nc.dram_tensor("bk", [128, HG], F32, kind="ExternalInput")
    bvb = nc.dram_tensor("bvb", [128, CG], BF16, kind="ExternalInput")
    wpT = nc.dram_tensor("wpT", [CG, C], BF16, kind="ExternalInput")
    maskT = nc.dram_tensor("maskT", [128, 16], F32, kind="ExternalInput")
    cdg01 = nc.dram_tensor("cdg01", [128, 4, 512], BF16, kind="ExternalInput")
    ones128 = nc.dram_tensor("ones128", [128, 128], BF16, kind="ExternalInput")
    out = nc.dram_tensor("out", [T, C], F32, kind="ExternalOutput")

    add = mybir.AluOpType.add
    mult = mybir.AluOpType.mult
    Exp = mybir.ActivationFunctionType.Exp
    Copy = mybir.ActivationFunctionType.Copy

    with TileContext(nc) as tc:
        # ---- constants that live for the whole kernel ----
        with tc.tile_pool(name="const", bufs=1) as cpool:
            maskT_sb = cpool.tile([128, 16], F32)
            nc.scalar.dma_start(out=maskT_sb, in_=maskT[:, :])
            cdg01_sb = cpool.tile([128, 4, 512], BF16)
            nc.scalar.dma_start(out=cdg01_sb, in_=cdg01[:, :, :])
            ones_sb = cpool.tile([128, 128], BF16)
            nc.scalar.dma_start(out=ones_sb, in_=ones128[:, :])
            bq_sb = cpool.tile([128, HG], F32)
            nc.scalar.dma_start(out=bq_sb, in_=bq[:, :])
            bk_sb = cpool.tile([128, HG], F32)
            nc.scalar.dma_start(out=bk_sb, in_=bk[:, :])
            bv_sb = cpool.tile([128, CG], BF16)
            nc.scalar.dma_start(out=bv_sb, in_=bvb[:, :])

            with tc.tile_pool(name="yt", bufs=1) as ytpool:
                yT_sb = ytpool.tile([128, HG, T], BF16)

                with (
                    tc.tile_pool(name="xx", bufs=1) as xpool,
                    tc.tile_pool(name="qk", bufs=2) as qkpool,
                    tc.tile_pool(name="vh", bufs=2) as vhpool,
                    tc.tile_pool(name="wv", bufs=2) as wvpool,
                    tc.tile_pool(name="wqk", bufs=2) as wqkpool,
                    tc.tile_pool(name="pt", bufs=2) as ptpool,
                    tc.tile_pool(name="zz", bufs=2) as zpool,
                    tc.tile_pool(name="psqk", bufs=2, space="PSUM") as psqk,
                    tc.tile_pool(name="psv", bufs=2, space="PSUM") as psv,
                    tc.tile_pool(name="psst", bufs=3, space="PSUM") as psst,
                    tc.tile_pool(name="psy", bufs=1, space="PSUM") as psy,
                ):
                    # ---------- startup DMAs across 4 queues ----------
                    # x loaded in t-slices so the tr-major prologue can start
                    # on slice 0 while the rest stream in
                    xt = xpool.tile([128, 16, T], BF16)
                    xq = [nc.sync, nc.scalar, nc.gpsimd, nc.sync]
                    for ts in range(4):
                        xq[ts].dma_start(
                            out=xt[:, :, ts * 512:(ts + 1) * 512],
                            in_=xT[:, ts * 512:(ts + 1) * 512].rearrange(
                                "(cc p) t -> p cc t", p=128
                            ),
                        )

                    wq_t = {}   # (head, 'q'/'k') -> weight tile
                    wv_t = {}   # pair -> weight tile
                    q_ring = {}
                    k_ring = {}
                    vh_ring = {}

                    def dma_wqk(dc, which, queue):
                        w_dram = wqT if which == "q" else wkT
                        wt = wqkpool.tile([128, 16, 128], BF16,
                                          tag=f"w{which}",
                                          name=f"w{which}{dc}")
                        queue.dma_start(
                            out=wt,
                            in_=w_dram[:, dc * 128:(dc + 1) * 128].rearrange(
                                "(cc p) d -> p cc d", p=128
                            ),
                        )
                        wq_t[(dc, which)] = wt

                    def dma_wv(p, queue):
                        wt = wvpool.tile([128, 16, 256], BF16, tag="wv",
                                         name=f"wv{p}")
                        queue.dma_start(
                            out=wt,
                            in_=wvT[:, p * 256:(p + 1) * 256].rearrange(
                                "(cc p) d -> p cc d", p=128
                            ),
                        )
                        wv_t[p] = wt

                    def qk_quarter(dc, which, tr):
                        """Thunks for 16 matmuls + 1 bias drain: one 512-t
                        quarter of q or k for head dc."""
                        ring = q_ring if which == "q" else k_ring
                        if dc not in ring:
                            ring[dc] = qkpool.tile(
                                [128, T], BF16, tag=f"{which}ring",
                                name=f"{which}{dc}",
                            )
                        wt = wq_t[(dc, which)]
                        b_sb = bq_sb if which == "q" else bk_sb
                        ps = psqk.tile([128, 512], F32, tag="qkps",
                                       name="qkps")

                        def mm(cc):
                            nc.tensor.matmul(
                                ps,
                                wt[:, cc, :],
                                xt[:, cc, tr * 512:(tr + 1) * 512],
                                start=(cc == 0),
                                stop=(cc == 15),
                            )

                        def drain():
                            nc.vector.tensor_scalar_add(
                                ring[dc][:, tr * 512:(tr + 1) * 512],
                                ps, b_sb[:, dc:dc + 1],
                            )

                        return [lambda cc=cc: mm(cc) for cc in range(16)] + \
                            [drain]

                    def v_group(p, tcb):
                        """Thunks for 16 matmuls (N=256) + bias drain: one
                        t-block of v for head pair p."""
                        if p not in vh_ring:
                            vh_ring[p] = vhpool.tile(
                                [128, 16, 256], BF16, tag="vh", name=f"vh{p}",
                            )
                        ps = psv.tile([128, 256], F32, tag="vps", name="vps")

                        def mm(cc):
                            nc.tensor.matmul(
                                ps,
                                xt[:, cc, tcb * 128:(tcb + 1) * 128],
                                wv_t[p][:, cc, :],
                                start=(cc == 0),
                                stop=(cc == 15),
                            )

                        def drain():
                            nc.vector.tensor_tensor(
                                vh_ring[p][:, tcb, :], ps,
                                bv_sb[:, p * 256:(p + 1) * 256], add,
                            )

                        return [lambda cc=cc: mm(cc) for cc in range(16)] + \
                            [drain]

                    # ---------- prologue ----------
                    dma_wqk(0, "q", nc.sync)
                    dma_wqk(0, "k", nc.scalar)
                    dma_wv(0, nc.scalar)
                    dma_wv(1, nc.gpsimd)
                    dma_wqk(1, "q", nc.sync)
                    dma_wqk(1, "k", nc.gpsimd)

                    # tr-major so compute starts on x t-slice 0 immediately
                    for tr in range(4):
                        for t in qk_quarter(0, "q", tr):
                            t()
                        for t in qk_quarter(0, "k", tr):
                            t()
                        for tcb in range(4 * tr, 4 * tr + 4):
                            for t in v_group(0, tcb):
                                t()

                    # qk projection schedule: step -> [(dc, which, tr)].
                    # Head 7's k quarters 2/3 are deferred into slot 7 so its
                    # steps have PE filler.
                    qk_sched = {}
                    for hh in range(6):
                        for cc_ in range(4):
                            qk_sched[4 * hh + cc_] = [
                                (hh + 1, "q", cc_), (hh + 1, "k", cc_)]
                    qk_sched[24] = [(7, "q", 0), (7, "k", 0)]
                    qk_sched[25] = [(7, "q", 1), (7, "k", 1)]
                    qk_sched[26] = [(7, "q", 2)]
                    qk_sched[27] = [(7, "q", 3)]
                    qk_sched[28] = [(7, "k", 2)]
                    qk_sched[29] = [(7, "k", 3)]

                    # ---------- fused head/chunk steps ----------
                    # per-chunk state kept across steps for the s-1 tail
                    state = {}

                    def av_thunks(h, c, pt, njb, za, zb):
                        """Thunks: AV accumulation, Z broadcast matmuls, then
                        1/Z + yT normalize (DVE) for chunk (h, c)."""
                        vh = vh_ring[h // 2]
                        dlo = (h % 2) * 128
                        yps = psy.tile([128, 512], F32, tag="y", name="y")
                        zbc = psv.tile([128, 512], F32, tag="vps", name="zbc")

                        def av_mm(jb):
                            sdg = jb - 4 * c
                            lo = sdg * 128 if sdg > 0 else 0
                            nc.tensor.matmul(
                                yps[:, lo:512],
                                vh[:, jb, dlo:dlo + 128],
                                pt[:, jb, lo:512],
                                start=(jb == 0),
                                stop=(jb == njb - 1),
                            )

                        def zm_a():
                            nc.tensor.matmul(zbc, ones_sb, za, start=True,
                                             stop=False)

                        def zm_b_norm():
                            nc.tensor.matmul(zbc, ones_sb, zb, start=False,
                                             stop=True)
                            rr = zpool.tile([128, 512], F32, tag="rr",
                                            name="rr")
                            nc.vector.reciprocal_approx_fast(out=rr, in_=zbc)
                            nc.vector.tensor_tensor(
                                yT_sb[:, h, c * 512:(c + 1) * 512], yps, rr,
                                mult,
                            )

                        return [lambda jb=jb: av_mm(jb)
                                for jb in range(njb)] + [zm_a, zm_b_norm]

                    for s in range(33):
                        # ---- build this step's filler (prev chunk tail +
                        # projection GEMMs) ----
                        filler = []
                        if s >= 1:
                            hp, cp = divmod(s - 1, 4)
                            ptp, njbp, zap, zbp = state.pop((hp, cp))
                            filler += av_thunks(hp, cp, ptp, njbp, zap, zbp)
                        if s < 32:
                            h, c = divmod(s, 4)
                            # weight prefetch for upcoming work
                            if c == 0 and h + 2 < HG:
                                dma_wqk(h + 2, "q", nc.sync)
                                dma_wqk(h + 2, "k", nc.gpsimd)
                            if c == 2 and h % 2 == 1:
                                p = (h + 3) // 2
                                if p <= 3:
                                    dma_wv(p, nc.gpsimd)
                            for (dc_, which_, tr_) in qk_sched.get(s, []):
                                filler += qk_quarter(dc_, which_, tr_)
                            p = h // 2 + 1
                            if p <= 3:
                                tb = (h % 2) * 8 + c * 2
                                filler += v_group(p, tb)
                                filler += v_group(p, tb + 1)

                        fi = iter(filler)

                        def pull(n):
                            for _ in range(n):
                                t = next(fi, None)
                                if t is None:
                                    return
                                t()

                        # ---- S blocks + exp, interleaved with filler ----
                        if s < 32:
                            njb = 4 * (c + 1)
                            pt = ptpool.tile([128, 16, 512], BF16, tag="pt")
                            # clear the stale [0, lo) regions of the diagonal
                            # blocks (read by the staircase multiply / AV)
                            for sdg in range(1, 4):
                                nc.vector.memset(
                                    pt[:, 4 * c + sdg, 0:sdg * 128], 0.0)
                            for jb in range(njb):
                                sdg = jb - 4 * c
                                lo = sdg * 128 if sdg > 0 else 0
                                ps = psst.tile([128, 512], F32, tag="s",
                                               name="s")
                                nc.tensor.matmul(
                                    ps[:, lo:512],
                                    k_ring[h][:, jb * 128:(jb + 1) * 128],
                                    q_ring[h][:, c * 512 + lo:(c + 1) * 512],
                                    start=True,
                                    stop=True,
                                )
                                nc.scalar.activation(
                                    pt[:, jb, lo:512], ps[:, lo:512], Exp,
                                    bias=maskT_sb[:, jb:jb + 1],
                                )
                                pull(3)
                            # staircase zero of the diagonal blocks
                            nc.vector.tensor_tensor(
                                pt[:, 4 * c:4 * c + 4, :],
                                pt[:, 4 * c:4 * c + 4, :],
                                cdg01_sb[:, :, :], mult,
                            )
                            # pairwise partial-sum tree -> za, zb (bf16)
                            za = zpool.tile([128, 512], BF16, tag="za",
                                            name="za")
                            zb = zpool.tile([128, 512], BF16, tag="zb",
                                            name="zb")
                            nc.vector.tensor_tensor(za, pt[:, 0, :],
                                                    pt[:, 2, :], add)
                            nc.vector.tensor_tensor(zb, pt[:, 1, :],
                                                    pt[:, 3, :], add)
                            for base in range(4, njb, 2):
                                nc.vector.tensor_tensor(
                                    za, za, pt[:, base, :], add)
                                nc.vector.tensor_tensor(
                                    zb, zb, pt[:, base + 1, :], add)
                            state[(h, c)] = (pt, njb, za, zb)
                        # ---- flush remaining filler ----
                        pull(len(filler))

                # ---------- phase 3: out = yT^T @ WpT ----------
                with (
                    tc.tile_pool(name="wp", bufs=1) as wppool,
                    tc.tile_pool(name="p3ps", bufs=4, space="PSUM") as ps3,
                    tc.tile_pool(name="p3o", bufs=4) as op3,
                ):
                    wp_sb = wppool.tile([128, HG, C], BF16)
                    oq = [nc.sync, nc.gpsimd]
                    for hh in range(HG):
                        oq[hh % 2].dma_start(
                            out=wp_sb[:, hh, :],
                            in_=wpT[hh * 128:(hh + 1) * 128, :],
                        )
                    for tcb in range(16):
                        pss = [ps3.tile([128, 512], F32, tag="ps3",
                                        name=f"ps3_{cr}")
                               for cr in range(4)]
                        for hh in range(HG):
                            for cr in range(4):
                                nc.tensor.matmul(
                                    pss[cr],
                                    yT_sb[:, hh, tcb * 128:(tcb + 1) * 128],
                                    wp_sb[:, hh, cr * 512:(cr + 1) * 512],
                                    start=(hh == 0),
                                    stop=(hh == HG - 1),
                                )
                        for cr in range(4):
                            ob = op3.tile([128, 512], F32, tag="ob")
                            nc.scalar.activation(ob, pss[cr], Copy)
                            nc.sync.dma_start(
                                out=out[tcb * 128:(tcb + 1) * 128,
                                        cr * 512:(cr + 1) * 512],
                                in_=ob,
                            )
    nc.compile()
    return nc


def get_nc():
    global _NC_CACHE
    if _NC_CACHE is None:
        _NC_CACHE = _build_program()
    return _NC_CACHE


def prep_core_inputs(inputs):
    """Host-side sharding / layout prep: slice per (b, g), transpose to the
    layouts the device program wants, fold the 1/sqrt(d) softmax scale into
    Wq/bq."""
    f = lambda a: np.asarray(a, dtype=np.float32)
    bf = ml_dtypes.bfloat16
    x = f(inputs["x"])
    am = f(inputs["attn_mask"])
    Wq, bq_ = f(inputs["Wq"]), f(inputs["bq"])
    Wk, bk_ = f(inputs["Wk"]), f(inputs["bk"])
    Wv, bv_ = f(inputs["Wv"]), f(inputs["bv"])
    Wp = f(inputs["Wp"])
    scale = 1.0 / math.sqrt(D)

    # 0/1 staircase in S^T layout: for diagonal block s (0..3) of a 512-wide
    # query chunk, partition p = key offset within the 128-block, column
    # i_local in [0, 512): masked (dead) iff i_local < s*128 + p.
    ii = np.arange(512)[None, :]
    pp = np.arange(128)[:, None]
    cdg01_t = np.stack(
        [np.where(ii < s * 128 + pp, 0.0, 1.0) for s in range(4)], axis=1
    ).astype(bf)  # [128, 4, 512]

    per_g = []
    for g in range(2):
        sl = slice(g * CG, (g + 1) * CG)
        per_g.append(dict(
            wqT=(np.ascontiguousarray(Wq[sl].T) * scale).astype(bf),
            wkT=np.ascontiguousarray(Wk[sl].T).astype(bf),
            wvT=np.ascontiguousarray(Wv[sl].T).astype(bf),
            bq=np.ascontiguousarray((bq_[sl] * scale).reshape(HG, 128).T),
            bk=np.ascontiguousarray(bk_[sl].reshape(HG, 128).T),
            bvb=np.ascontiguousarray(
                np.broadcast_to(bv_[sl], (128, CG))
            ).astype(bf),
            wpT=np.ascontiguousarray(Wp[:, sl].T).astype(bf),
        ))

    ones_t = np.ones((128, 128), dtype=bf)

    in_maps = []
    for core in range(N_CORES):
        b, g = core // 2, core % 2
        m = dict(per_g[g])
        m["xT"] = np.ascontiguousarray(x[b].T).astype(bf)
        m["maskT"] = np.ascontiguousarray(
            am[b, 0, 0, :].reshape(16, 128).T
        )
        m["cdg01"] = cdg01_t
        m["ones128"] = ones_t
        in_maps.append(m)
    return in_maps


def run(inputs, trace=False):
    nc = get_nc()
    in_maps = prep_core_inputs(inputs)
    rr = run_bass_kernel_spmd(nc, in_maps, list(range(N_CORES)), trace=trace)
    bp = np.asarray(inputs["bp"], dtype=np.float32)
    y = np.empty((B, T, C), dtype=np.float32)
    for b in range(B):
        y[b] = rr.results[2 * b]["out"] + rr.results[2 * b + 1]["out"] + bp[None, :]
    return y, rr


def kernel(**inputs):
    y, _ = run(inputs)
    return y


# revision 23
# speedup vs baseline: 1.6416x; 1.0301x over previous
"""Causal self-attention (B=4, T=2048, C=2048, H=16) on 8 NeuronCores.

Sharding: core c = (b, g) with b = c // 2 (batch), g = c % 2 (head group of 8
heads = 1024 channels). Data parallel over B, tensor parallel over heads; the
output projection is computed per head-group and the two partials per batch
are summed on the host (+ bp).

Device program: a fused per-head pipeline. The QKV projections for head h+1
(pure GEMM) are interleaved into head h's attention steps so the scalar
(exp) and vector (sums/normalize) work hides under tensor-engine GEMMs and
the PE never idles long enough to re-throttle (HAM).

Per chunk step s = (h, c) with njb = 4(c+1) key blocks:
  PE:  S^T blocks (keys on partitions)  ->  AV accumulation of chunk s-1
       -> Z broadcast matmuls (ones128^T @ za/zb) -> projection GEMM slice
       (q/k for head h+1, v for head-pair h//2+1)
  ACT: exp of each S block with the additive attn mask as per-partition bias
  DVE: 1/Z (128-wide, in SBUF), yT normalize mult of chunk s-1, causal
       staircase zeroing (one bf16 0/1 multiply per chunk), the za/zb
       pairwise partial-sum tree, projection bias adds.
v stays resident in SBUF (no DRAM round trip); Z broadcast comes straight
from a [128,128] ones stationary so there is no transpose/DRAM bounce on the
PE critical path. Phase 3 (out = yT^T Wp) streams Wp after the slots finish.
"""

import math

import numpy as np
import ml_dtypes

import concourse.bass as bass
import concourse.bacc as bacc
import concourse.mybir as mybir
from concourse.tile import TileContext
from concourse.bass_utils import run_bass_kernel_spmd

T = 2048
C = 2048
N_HEAD = 16
D = 128          # head dim
HG = 8           # heads per core
CG = HG * D      # 1024: per-core projection width
B = 4
N_CORES = 8

F32 = mybir.dt.float32
BF16 = mybir.dt.bfloat16

_NC_CACHE = None


def _build_program():
    nc = bacc.Bacc("TRN2", target_bir_lowering=False, debug=False)

    xT = nc.dram_tensor("xT", [C, T], BF16, kind="ExternalInput")
    wqT = nc.dram_tensor("wqT", [C, CG], BF16, kind="ExternalInput")
    wkT = nc.dram_tensor("wkT", [C, CG], BF16, kind="ExternalInput")
    wvT = nc.dram_tensor("wvT", [C, CG], BF16, kind="ExternalInput")
    bq = nc.dram_tensor("bq", [128, HG], F32, kind="ExternalInput")
    bk = nc.dram_tensor("bk", [128, HG], F32, kind="ExternalInput")
    bvb = nc.dram_tensor("bvb", [128, CG], BF16, kind="ExternalInput")
    wpT = nc.dram_tensor("wpT", [CG, C], BF16, kind="ExternalInput")
    maskT = nc.dram_tensor("maskT", [128, 16], F32, kind="ExternalInput")
    cdg01 = nc.dram_tensor("cdg01", [128, 4, 512], BF16, kind="ExternalInput")
    ones128 = nc.dram_tensor("ones128", [128, 128], BF16, kind="ExternalInput")
    out = nc.dram_tensor("out", [T, C], BF16, kind="ExternalOutput")

    add = mybir.AluOpType.add
    mult = mybir.AluOpType.mult
    Exp = mybir.ActivationFunctionType.Exp
    Copy = mybir.ActivationFunctionType.Copy

    with TileContext(nc) as tc:
        # ---- constants that live for the whole kernel ----
        with tc.tile_pool(name="const", bufs=1) as cpool:
            maskT_sb = cpool.tile([128, 16], F32)
            nc.scalar.dma_start(out=maskT_sb, in_=maskT[:, :])
            cdg01_sb = cpool.tile([128, 4, 512], BF16)
            nc.scalar.dma_start(out=cdg01_sb, in_=cdg01[:, :, :])
            ones_sb = cpool.tile([128, 128], BF16)
            nc.scalar.dma_start(out=ones_sb, in_=ones128[:, :])
            bq_sb = cpool.tile([128, HG], F32)
            nc.scalar.dma_start(out=bq_sb, in_=bq[:, :])
            bk_sb = cpool.tile([128, HG], F32)
            nc.scalar.dma_start(out=bk_sb, in_=bk[:, :])
            bv_sb = cpool.tile([128, CG], BF16)
            nc.scalar.dma_start(out=bv_sb, in_=bvb[:, :])

            with tc.tile_pool(name="yt", bufs=1) as ytpool:
                yT_sb = ytpool.tile([128, HG, T], BF16)

                with (
                    tc.tile_pool(name="xx", bufs=1) as xpool,
                    tc.tile_pool(name="qk", bufs=2) as qkpool,
                    tc.tile_pool(name="vh", bufs=2) as vhpool,
                    tc.tile_pool(name="wv", bufs=2) as wvpool,
                    tc.tile_pool(name="wqk", bufs=2) as wqkpool,
                    tc.tile_pool(name="pt", bufs=2) as ptpool,
                    tc.tile_pool(name="zz", bufs=2) as zpool,
                    tc.tile_pool(name="psqk", bufs=2, space="PSUM") as psqk,
                    tc.tile_pool(name="psv", bufs=2, space="PSUM") as psv,
                    tc.tile_pool(name="psst", bufs=3, space="PSUM") as psst,
                    tc.tile_pool(name="psy", bufs=1, space="PSUM") as psy,
                ):
                    # ---------- startup DMAs across 4 queues ----------
                    # x loaded in t-slices so the tr-major prologue can start
                    # on slice 0 while the rest stream in. Slice 0 goes first
                    # on an otherwise-empty queue, split in cc halves so the
                    # first projection matmuls can start on the first half.
                    xt = xpool.tile([128, 16, T], BF16)

                    def dma_x(ts, clo, chi, queue):
                        queue.dma_start(
                            out=xt[:, clo:chi, ts * 512:(ts + 1) * 512],
                            in_=xT[clo * 128:chi * 128,
                                   ts * 512:(ts + 1) * 512].rearrange(
                                "(cc p) t -> p cc t", p=128
                            ),
                        )

                    dma_x(0, 0, 8, nc.sync)
                    dma_x(0, 8, 16, nc.sync)
                    dma_x(1, 0, 16, nc.scalar)

                    wq_t = {}   # (head, 'q'/'k') -> weight tile
                    wv_t = {}   # pair -> weight tile
                    q_ring = {}
                    k_ring = {}
                    vh_ring = {}

                    def dma_wqk(dc, which, queue):
                        w_dram = wqT if which == "q" else wkT
                        wt = wqkpool.tile([128, 16, 128], BF16,
                                          tag=f"w{which}",
                                          name=f"w{which}{dc}")
                        queue.dma_start(
                            out=wt,
                            in_=w_dram[:, dc * 128:(dc + 1) * 128].rearrange(
                                "(cc p) d -> p cc d", p=128
                            ),
                        )
                        wq_t[(dc, which)] = wt

                    def dma_wv(p, queue):
                        wt = wvpool.tile([128, 16, 256], BF16, tag="wv",
                                         name=f"wv{p}")
                        queue.dma_start(
                            out=wt,
                            in_=wvT[:, p * 256:(p + 1) * 256].rearrange(
                                "(cc p) d -> p cc d", p=128
                            ),
                        )
                        wv_t[p] = wt

                    def qk_quarter(dc, which, tr):
                        """Thunks for 16 matmuls + 1 bias drain: one 512-t
                        quarter of q or k for head dc."""
                        ring = q_ring if which == "q" else k_ring
                        if dc not in ring:
                            ring[dc] = qkpool.tile(
                                [128, T], BF16, tag=f"{which}ring",
                                name=f"{which}{dc}",
                            )
                        wt = wq_t[(dc, which)]
                        b_sb = bq_sb if which == "q" else bk_sb
                        ps = psqk.tile([128, 512], F32, tag="qkps",
                                       name="qkps")

                        def mm(cc):
                            nc.tensor.matmul(
                                ps,
                                wt[:, cc, :],
                                xt[:, cc, tr * 512:(tr + 1) * 512],
                                start=(cc == 0),
                                stop=(cc == 15),
                            )

                        def drain():
                            nc.vector.tensor_scalar_add(
                                ring[dc][:, tr * 512:(tr + 1) * 512],
                                ps, b_sb[:, dc:dc + 1],
                            )

                        return [lambda cc=cc: mm(cc) for cc in range(16)] + \
                            [drain]

                    def v_group(p, tcb):
                        """Thunks for 16 matmuls (N=256) + bias drain: one
                        t-block of v for head pair p."""
                        if p not in vh_ring:
                            vh_ring[p] = vhpool.tile(
                                [128, 16, 256], BF16, tag="vh", name=f"vh{p}",
                            )
                        ps = psv.tile([128, 256], F32, tag="vps", name="vps")

                        def mm(cc):
                            nc.tensor.matmul(
                                ps,
                                xt[:, cc, tcb * 128:(tcb + 1) * 128],
                                wv_t[p][:, cc, :],
                                start=(cc == 0),
                                stop=(cc == 15),
                            )

                        def drain():
                            nc.vector.tensor_tensor(
                                vh_ring[p][:, tcb, :], ps,
                                bv_sb[:, p * 256:(p + 1) * 256], add,
                            )

                        return [lambda cc=cc: mm(cc) for cc in range(16)] + \
                            [drain]

                    # ---------- prologue ----------
                    dma_wqk(0, "q", nc.gpsimd)
                    dma_wqk(0, "k", nc.gpsimd)
                    dma_wv(0, nc.gpsimd)
                    dma_x(2, 0, 16, nc.gpsimd)
                    dma_x(3, 0, 16, nc.sync)
                    dma_wqk(1, "q", nc.sync)
                    dma_wqk(1, "k", nc.scalar)
                    dma_wv(1, nc.gpsimd)

                    # tr-major so compute starts on x t-slice 0 immediately
                    for tr in range(4):
                        for t in qk_quarter(0, "q", tr):
                            t()
                        for t in qk_quarter(0, "k", tr):
                            t()
                        for tcb in range(4 * tr, 4 * tr + 4):
                            for t in v_group(0, tcb):
                                t()

                    # qk projection schedule: step -> [(dc, which, tr)].
                    # Head 7's k quarters 2/3 are deferred into slot 7 so its
                    # steps have PE filler.
                    qk_sched = {}
                    for hh in range(6):
                        for cc_ in range(4):
                            qk_sched[4 * hh + cc_] = [
                                (hh + 1, "q", cc_), (hh + 1, "k", cc_)]
                    qk_sched[24] = [(7, "q", 0), (7, "k", 0)]
                    qk_sched[25] = [(7, "q", 1), (7, "k", 1)]
                    qk_sched[26] = [(7, "q", 2)]
                    qk_sched[27] = [(7, "q", 3)]
                    qk_sched[28] = [(7, "k", 2)]
                    qk_sched[29] = [(7, "k", 3)]

                    # ---------- fused head/chunk steps ----------
                    # per-chunk state kept across steps for the s-1 tail
                    state = {}

                    def av_thunks(h, c, pt, njb, za, zb):
                        """Thunks: AV accumulation, Z broadcast matmuls, then
                        1/Z + yT normalize (DVE) for chunk (h, c)."""
                        vh = vh_ring[h // 2]
                        dlo = (h % 2) * 128
                        yps = psy.tile([128, 512], F32, tag="y", name="y")
                        zbc = psv.tile([128, 512], F32, tag="vps", name="zbc")

                        def av_mm(jb):
                            sdg = jb - 4 * c
                            lo = sdg * 128 if sdg > 0 else 0
                            nc.tensor.matmul(
                                yps[:, lo:512],
                                vh[:, jb, dlo:dlo + 128],
                                pt[:, jb, lo:512],
                                start=(jb == 0),
                                stop=(jb == njb - 1),
                            )

                        def zm_a():
                            nc.tensor.matmul(zbc, ones_sb, za, start=True,
                                             stop=False)

                        def zm_b_norm():
                            nc.tensor.matmul(zbc, ones_sb, zb, start=False,
                                             stop=True)
                            rr = zpool.tile([128, 512], F32, tag="rr",
                                            name="rr")
                            nc.vector.reciprocal_approx_fast(out=rr, in_=zbc)
                            nc.vector.tensor_tensor(
                                yT_sb[:, h, c * 512:(c + 1) * 512], yps, rr,
                                mult,
                            )

                        return [lambda jb=jb: av_mm(jb)
                                for jb in range(njb)] + [zm_a, zm_b_norm]

                    for s in range(33):
                        # ---- build this step's filler (prev chunk tail +
                        # projection GEMMs) ----
                        filler = []
                        if s >= 1:
                            hp, cp = divmod(s - 1, 4)
                            ptp, njbp, zap, zbp = state.pop((hp, cp))
                            filler += av_thunks(hp, cp, ptp, njbp, zap, zbp)
                        if s < 32:
                            h, c = divmod(s, 4)
                            # weight prefetch for upcoming work
                            if c == 0 and h + 2 < HG:
                                dma_wqk(h + 2, "q", nc.sync)
                                dma_wqk(h + 2, "k", nc.gpsimd)
                            if c == 2 and h % 2 == 1:
                                p = (h + 3) // 2
                                if p <= 3:
                                    dma_wv(p, nc.gpsimd)
                            for (dc_, which_, tr_) in qk_sched.get(s, []):
                                filler += qk_quarter(dc_, which_, tr_)
                            p = h // 2 + 1
                            if p <= 3:
                                tb = (h % 2) * 8 + c * 2
                                filler += v_group(p, tb)
                                filler += v_group(p, tb + 1)

                        fi = iter(filler)

                        def pull(n):
                            for _ in range(n):
                                t = next(fi, None)
                                if t is None:
                                    return
                                t()

                        # ---- S blocks + exp, interleaved with filler ----
                        if s < 32:
                            njb = 4 * (c + 1)
                            pt = ptpool.tile([128, 16, 512], BF16, tag="pt")
                            # clear the stale [0, lo) regions of the diagonal
                            # blocks (read by the staircase multiply / AV)
                            for sdg in range(1, 4):
                                nc.vector.memset(
                                    pt[:, 4 * c + sdg, 0:sdg * 128], 0.0)
                            for jb in range(njb):
                                sdg = jb - 4 * c
                                lo = sdg * 128 if sdg > 0 else 0
                                ps = psst.tile([128, 512], F32, tag="s",
                                               name="s")
                                nc.tensor.matmul(
                                    ps[:, lo:512],
                                    k_ring[h][:, jb * 128:(jb + 1) * 128],
                                    q_ring[h][:, c * 512 + lo:(c + 1) * 512],
                                    start=True,
                                    stop=True,
                                )
                                nc.scalar.activation(
                                    pt[:, jb, lo:512], ps[:, lo:512], Exp,
                                    bias=maskT_sb[:, jb:jb + 1],
                                )
                                pull(3)
                            # staircase zero of the diagonal blocks
                            nc.vector.tensor_tensor(
                                pt[:, 4 * c:4 * c + 4, :],
                                pt[:, 4 * c:4 * c + 4, :],
                                cdg01_sb[:, :, :], mult,
                            )
                            # pairwise partial-sum tree -> za, zb (bf16)
                            za = zpool.tile([128, 512], BF16, tag="za",
                                            name="za")
                            zb = zpool.tile([128, 512], BF16, tag="zb",
                                            name="zb")
                            nc.vector.tensor_tensor(za, pt[:, 0, :],
                                                    pt[:, 2, :], add)
                            nc.vector.tensor_tensor(zb, pt[:, 1, :],
                                                    pt[:, 3, :], add)
                            for base in range(4, njb, 2):
                                nc.vector.tensor_tensor(
                                    za, za, pt[:, base, :], add)
                                nc.vector.tensor_tensor(
                                    zb, zb, pt[:, base + 1, :], add)
                            state[(h, c)] = (pt, njb, za, zb)
                        # ---- flush remaining filler ----
                        pull(len(filler))

                # ---------- phase 3: out = yT^T @ WpT ----------
                with (
                    tc.tile_pool(name="wp", bufs=1) as wppool,
                    tc.tile_pool(name="p3ps", bufs=4, space="PSUM") as ps3,
                    tc.tile_pool(name="p3o", bufs=4) as op3,
                ):
                    wp_sb = wppool.tile([128, HG, C], BF16)
                    oq = [nc.sync, nc.gpsimd]
                    for hh in range(HG):
                        oq[hh % 2].dma_start(
                            out=wp_sb[:, hh, :],
                            in_=wpT[hh * 128:(hh + 1) * 128, :],
                        )
                    for tcb in range(16):
                        pss = [ps3.tile([128, 512], F32, tag="ps3",
                                        name=f"ps3_{cr}")
                               for cr in range(4)]
                        for hh in range(HG):
                            for cr in range(4):
                                nc.tensor.matmul(
                                    pss[cr],
                                    yT_sb[:, hh, tcb * 128:(tcb + 1) * 128],
                                    wp_sb[:, hh, cr * 512:(cr + 1) * 512],
                                    start=(hh == 0),
                                    stop=(hh == HG - 1),
                                )
                        for cr in range(4):
                            ob = op3.tile([128, 512], BF16, tag="ob")
                            nc.scalar.activation(ob, pss[cr], Copy)
                            (nc.sync if cr % 2 == 0 else nc.gpsimd).dma_start(
                                out=out[tcb * 128:(tcb + 1) * 128,
                                        cr * 512:(cr + 1) * 512],
                                in_=ob,
                            )
    nc.compile()
    return nc


def get_nc():
    global _NC_CACHE
    if _NC_CACHE is None:
        _NC_CACHE = _build_program()
    return _NC_CACHE


def prep_core_inputs(inputs):
    """Host-side sharding / layout prep: slice per (b, g), transpose to the
    layouts the device program wants, fold the 1/sqrt(d) softmax scale into
    Wq/bq."""
    f = lambda a: np.asarray(a, dtype=np.float32)
    bf = ml_dtypes.bfloat16
    x = f(inputs["x"])
    am = f(inputs["attn_mask"])
    Wq, bq_ = f(inputs["Wq"]), f(inputs["bq"])
    Wk, bk_ = f(inputs["Wk"]), f(inputs["bk"])
    Wv, bv_ = f(inputs["Wv"]), f(inputs["bv"])
    Wp = f(inputs["Wp"])
    scale = 1.0 / math.sqrt(D)

    # 0/1 staircase in S^T layout: for diagonal block s (0..3) of a 512-wide
    # query chunk, partition p = key offset within the 128-block, column
    # i_local in [0, 512): masked (dead) iff i_local < s*128 + p.
    ii = np.arange(512)[None, :]
    pp = np.arange(128)[:, None]
    cdg01_t = np.stack(
        [np.where(ii < s * 128 + pp, 0.0, 1.0) for s in range(4)], axis=1
    ).astype(bf)  # [128, 4, 512]

    per_g = []
    for g in range(2):
        sl = slice(g * CG, (g + 1) * CG)
        per_g.append(dict(
            wqT=(np.ascontiguousarray(Wq[sl].T) * scale).astype(bf),
            wkT=np.ascontiguousarray(Wk[sl].T).astype(bf),
            wvT=np.ascontiguousarray(Wv[sl].T).astype(bf),
            bq=np.ascontiguousarray((bq_[sl] * scale).reshape(HG, 128).T),
            bk=np.ascontiguousarray(bk_[sl].reshape(HG, 128).T),
            bvb=np.ascontiguousarray(
                np.broadcast_to(bv_[sl], (128, CG))
            ).astype(bf),
            wpT=np.ascontiguousarray(Wp[:, sl].T).astype(bf),
        ))

    ones_t = np.ones((128, 128), dtype=bf)

    in_maps = []
    for core in range(N_CORES):
        b, g = core // 2, core % 2
        m = dict(per_g[g])
        m["xT"] = np.ascontiguousarray(x[b].T).astype(bf)
        m["maskT"] = np.ascontiguousarray(
            am[b, 0, 0, :].reshape(16, 128).T
        )
        m["cdg01"] = cdg01_t
        m["ones128"] = ones_t
        in_maps.append(m)
    return in_maps


def run(inputs, trace=False):
    nc = get_nc()
    in_maps = prep_core_inputs(inputs)
    rr = run_bass_kernel_spmd(nc, in_maps, list(range(N_CORES)), trace=trace)
    bp = np.asarray(inputs["bp"], dtype=np.float32)
    y = np.empty((B, T, C), dtype=np.float32)
    for b in range(B):
        y[b] = (np.asarray(rr.results[2 * b]["out"], dtype=np.float32)
                + np.asarray(rr.results[2 * b + 1]["out"], dtype=np.float32)
                + bp[None, :])
    return y, rr


def kernel(**inputs):
    y, _ = run(inputs)
    return y


# revision 25
# speedup vs baseline: 1.6418x; 1.0001x over previous
"""Causal self-attention (B=4, T=2048, C=2048, H=16) on 8 NeuronCores.

Sharding: core c = (b, g) with b = c // 2 (batch), g = c % 2 (head group of 8
heads = 1024 channels). Data parallel over B, tensor parallel over heads; the
output projection is computed per head-group and the two partials per batch
are summed on the host (+ bp).

Device program: a fused per-head pipeline. The QKV projections for head h+1
(pure GEMM) are interleaved into head h's attention steps so the scalar
(exp) and vector (sums/normalize) work hides under tensor-engine GEMMs and
the PE never idles long enough to re-throttle (HAM).

Per chunk step s = (h, c) with njb = 4(c+1) key blocks:
  PE:  S^T blocks (keys on partitions)  ->  AV accumulation of chunk s-1
       -> Z broadcast matmuls (ones128^T @ za/zb) -> projection GEMM slice
       (q/k for head h+1, v for head-pair h//2+1)
  ACT: exp of each S block with the additive attn mask as per-partition bias
  DVE: 1/Z (128-wide, in SBUF), yT normalize mult of chunk s-1, causal
       staircase zeroing (one bf16 0/1 multiply per chunk), the za/zb
       pairwise partial-sum tree, projection bias adds.
v stays resident in SBUF (no DRAM round trip); Z broadcast comes straight
from a [128,128] ones stationary so there is no transpose/DRAM bounce on the
PE critical path. Phase 3 (out = yT^T Wp) streams Wp after the slots finish.
"""

import math

import numpy as np
import ml_dtypes

import concourse.bass as bass
import concourse.bacc as bacc
import concourse.mybir as mybir
from concourse.tile import TileContext
from concourse.bass_utils import run_bass_kernel_spmd

T = 2048
C = 2048
N_HEAD = 16
D = 128          # head dim
HG = 8           # heads per core
CG = HG * D      # 1024: per-core projection width
B = 4
N_CORES = 8

F32 = mybir.dt.float32
BF16 = mybir.dt.bfloat16

_NC_CACHE = None


def _build_program():
    nc = bacc.Bacc("TRN2", target_bir_lowering=False, debug=False)

    xT = nc.dram_tensor("xT", [C, T], BF16, kind="ExternalInput")
    wqT = nc.dram_tensor("wqT", [C, CG], BF16, kind="ExternalInput")
    wkT = nc.dram_tensor("wkT", [C, CG], BF16, kind="ExternalInput")
    wvT = nc.dram_tensor("wvT", [C, CG], BF16, kind="ExternalInput")
    bq = nc.dram_tensor("bq", [128, HG], F32, kind="ExternalInput")
    bk = nc.dram_tensor("bk", [128, HG], F32, kind="ExternalInput")
    bvb = nc.dram_tensor("bvb", [128, CG], BF16, kind="ExternalInput")
    wpT = nc.dram_tensor("wpT", [CG, C], BF16, kind="ExternalInput")
    maskT = nc.dram_tensor("maskT", [128, 16], F32, kind="ExternalInput")
    cdg01 = nc.dram_tensor("cdg01", [128, 4, 512], BF16, kind="ExternalInput")
    ones128 = nc.dram_tensor("ones128", [128, 128], BF16, kind="ExternalInput")
    out = nc.dram_tensor("out", [T, C], BF16, kind="ExternalOutput")

    add = mybir.AluOpType.add
    mult = mybir.AluOpType.mult
    Exp = mybir.ActivationFunctionType.Exp
    Copy = mybir.ActivationFunctionType.Copy

    with TileContext(nc) as tc:
        # ---- constants that live for the whole kernel ----
        with tc.tile_pool(name="const", bufs=1) as cpool:
            maskT_sb = cpool.tile([128, 16], F32)
            nc.scalar.dma_start(out=maskT_sb, in_=maskT[:, :])
            cdg01_sb = cpool.tile([128, 4, 512], BF16)
            nc.scalar.dma_start(out=cdg01_sb, in_=cdg01[:, :, :])
            ones_sb = cpool.tile([128, 128], BF16)
            nc.scalar.dma_start(out=ones_sb, in_=ones128[:, :])
            bq_sb = cpool.tile([128, HG], F32)
            nc.scalar.dma_start(out=bq_sb, in_=bq[:, :])
            bk_sb = cpool.tile([128, HG], F32)
            nc.scalar.dma_start(out=bk_sb, in_=bk[:, :])
            bv_sb = cpool.tile([128, CG], BF16)
            nc.scalar.dma_start(out=bv_sb, in_=bvb[:, :])

            with tc.tile_pool(name="yt", bufs=1) as ytpool:
                yT_sb = ytpool.tile([128, HG, T], BF16)

                with (
                    tc.tile_pool(name="xx", bufs=1) as xpool,
                    tc.tile_pool(name="qk", bufs=2) as qkpool,
                    tc.tile_pool(name="vh", bufs=2) as vhpool,
                    tc.tile_pool(name="wv", bufs=2) as wvpool,
                    tc.tile_pool(name="wqk", bufs=2) as wqkpool,
                    tc.tile_pool(name="pt", bufs=2) as ptpool,
                    tc.tile_pool(name="zz", bufs=2) as zpool,
                    tc.tile_pool(name="psqk", bufs=2, space="PSUM") as psqk,
                    tc.tile_pool(name="psv", bufs=2, space="PSUM") as psv,
                    tc.tile_pool(name="psst", bufs=3, space="PSUM") as psst,
                    tc.tile_pool(name="psy", bufs=1, space="PSUM") as psy,
                ):
                    # ---------- startup DMAs across 4 queues ----------
                    # x loaded in t-slices so the tr-major prologue can start
                    # on slice 0 while the rest stream in. Slice 0 goes first
                    # on an otherwise-empty queue, split in cc halves so the
                    # first projection matmuls can start on the first half.
                    xt = xpool.tile([128, 16, T], BF16)

                    def dma_x(ts, clo, chi, queue):
                        queue.dma_start(
                            out=xt[:, clo:chi, ts * 512:(ts + 1) * 512],
                            in_=xT[clo * 128:chi * 128,
                                   ts * 512:(ts + 1) * 512].rearrange(
                                "(cc p) t -> p cc t", p=128
                            ),
                        )

                    dma_x(0, 0, 8, nc.sync)
                    dma_x(0, 8, 16, nc.sync)
                    dma_x(1, 0, 16, nc.sync)

                    wq_t = {}   # (head, 'q'/'k') -> weight tile
                    wv_t = {}   # pair -> weight tile
                    q_ring = {}
                    k_ring = {}
                    vh_ring = {}

                    def dma_wqk(dc, which, queue):
                        w_dram = wqT if which == "q" else wkT
                        wt = wqkpool.tile([128, 16, 128], BF16,
                                          tag=f"w{which}",
                                          name=f"w{which}{dc}")
                        queue.dma_start(
                            out=wt,
                            in_=w_dram[:, dc * 128:(dc + 1) * 128].rearrange(
                                "(cc p) d -> p cc d", p=128
                            ),
                        )
                        wq_t[(dc, which)] = wt

                    def dma_wv(p, queue):
                        wt = wvpool.tile([128, 16, 256], BF16, tag="wv",
                                         name=f"wv{p}")
                        queue.dma_start(
                            out=wt,
                            in_=wvT[:, p * 256:(p + 1) * 256].rearrange(
                                "(cc p) d -> p cc d", p=128
                            ),
                        )
                        wv_t[p] = wt

                    def qk_quarter(dc, which, tr):
                        """Thunks for 16 matmuls + 1 bias drain: one 512-t
                        quarter of q or k for head dc."""
                        ring = q_ring if which == "q" else k_ring
                        if dc not in ring:
                            ring[dc] = qkpool.tile(
                                [128, T], BF16, tag=f"{which}ring",
                                name=f"{which}{dc}",
                            )
                        wt = wq_t[(dc, which)]
                        b_sb = bq_sb if which == "q" else bk_sb
                        ps = psqk.tile([128, 512], F32, tag="qkps",
                                       name="qkps")

                        def mm(cc):
                            nc.tensor.matmul(
                                ps,
                                wt[:, cc, :],
                                xt[:, cc, tr * 512:(tr + 1) * 512],
                                start=(cc == 0),
                                stop=(cc == 15),
                            )

                        def drain():
                            nc.vector.tensor_scalar_add(
                                ring[dc][:, tr * 512:(tr + 1) * 512],
                                ps, b_sb[:, dc:dc + 1],
                            )

                        return [lambda cc=cc: mm(cc) for cc in range(16)] + \
                            [drain]

                    def v_group(p, tcb):
                        """Thunks for 16 matmuls (N=256) + bias drain: one
                        t-block of v for head pair p."""
                        if p not in vh_ring:
                            vh_ring[p] = vhpool.tile(
                                [128, 16, 256], BF16, tag="vh", name=f"vh{p}",
                            )
                        ps = psv.tile([128, 256], F32, tag="vps", name="vps")

                        def mm(cc):
                            nc.tensor.matmul(
                                ps,
                                xt[:, cc, tcb * 128:(tcb + 1) * 128],
                                wv_t[p][:, cc, :],
                                start=(cc == 0),
                                stop=(cc == 15),
                            )

                        def drain():
                            nc.vector.tensor_tensor(
                                vh_ring[p][:, tcb, :], ps,
                                bv_sb[:, p * 256:(p + 1) * 256], add,
                            )

                        return [lambda cc=cc: mm(cc) for cc in range(16)] + \
                            [drain]

                    # ---------- prologue ----------
                    dma_wqk(0, "q", nc.gpsimd)
                    dma_wqk(0, "k", nc.gpsimd)
                    dma_wv(0, nc.gpsimd)
                    dma_x(2, 0, 16, nc.gpsimd)
                    dma_wqk(1, "q", nc.scalar)
                    dma_x(3, 0, 16, nc.scalar)
                    dma_wv(1, nc.gpsimd)
                    dma_wqk(1, "k", nc.gpsimd)

                    # tr-major so compute starts on x t-slice 0 immediately;
                    # tr0 is interleaved at cc-half granularity so the first
                    # matmuls start on the first half of x slice 0
                    q0 = qk_quarter(0, "q", 0)
                    k0 = qk_quarter(0, "k", 0)
                    vg = [v_group(0, tcb) for tcb in range(4)]
                    for t in q0[0:8] + k0[0:8] + vg[0][0:8] + vg[1][0:8]:
                        t()
                    for t in q0[8:] + k0[8:] + vg[0][8:] + vg[1][8:]:
                        t()
                    for t in vg[2] + vg[3]:
                        t()
                    for tr in range(1, 4):
                        for t in qk_quarter(0, "q", tr):
                            t()
                        for t in qk_quarter(0, "k", tr):
                            t()
                        for tcb in range(4 * tr, 4 * tr + 4):
                            for t in v_group(0, tcb):
                                t()

                    # qk projection schedule: step -> [(dc, which, tr)].
                    # Head 7's k quarters 2/3 are deferred into slot 7 so its
                    # steps have PE filler.
                    qk_sched = {}
                    for hh in range(6):
                        for cc_ in range(4):
                            qk_sched[4 * hh + cc_] = [
                                (hh + 1, "q", cc_), (hh + 1, "k", cc_)]
                    qk_sched[24] = [(7, "q", 0), (7, "k", 0)]
                    qk_sched[25] = [(7, "q", 1), (7, "k", 1)]
                    qk_sched[26] = [(7, "q", 2)]
                    qk_sched[27] = [(7, "q", 3)]
                    qk_sched[28] = [(7, "k", 2)]
                    qk_sched[29] = [(7, "k", 3)]

                    # ---------- fused head/chunk steps ----------
                    # per-chunk state kept across steps for the s-1 tail
                    state = {}

                    def av_thunks(h, c, pt, njb, za, zb):
                        """Thunks: AV accumulation, Z broadcast matmuls, then
                        1/Z + yT normalize (DVE) for chunk (h, c)."""
                        vh = vh_ring[h // 2]
                        dlo = (h % 2) * 128
                        yps = psy.tile([128, 512], F32, tag="y", name="y")
                        zbc = psv.tile([128, 512], F32, tag="vps", name="zbc")

                        def av_mm(jb):
                            sdg = jb - 4 * c
                            lo = sdg * 128 if sdg > 0 else 0
                            nc.tensor.matmul(
                                yps[:, lo:512],
                                vh[:, jb, dlo:dlo + 128],
                                pt[:, jb, lo:512],
                                start=(jb == 0),
                                stop=(jb == njb - 1),
                            )

                        def zm_a():
                            nc.tensor.matmul(zbc, ones_sb, za, start=True,
                                             stop=False)

                        def zm_b_norm():
                            nc.tensor.matmul(zbc, ones_sb, zb, start=False,
                                             stop=True)
                            rr = zpool.tile([128, 512], F32, tag="rr",
                                            name="rr")
                            nc.vector.reciprocal_approx_fast(out=rr, in_=zbc)
                            nc.vector.tensor_tensor(
                                yT_sb[:, h, c * 512:(c + 1) * 512], yps, rr,
                                mult,
                            )

                        return [lambda jb=jb: av_mm(jb)
                                for jb in range(njb)] + [zm_a, zm_b_norm]

                    for s in range(33):
                        # ---- build this step's filler (prev chunk tail +
                        # projection GEMMs) ----
                        filler = []
                        if s >= 1:
                            hp, cp = divmod(s - 1, 4)
                            ptp, njbp, zap, zbp = state.pop((hp, cp))
                            filler += av_thunks(hp, cp, ptp, njbp, zap, zbp)
                        if s < 32:
                            h, c = divmod(s, 4)
                            # weight prefetch for upcoming work
                            if c == 0 and h + 2 < HG:
                                dma_wqk(h + 2, "q", nc.sync)
                                dma_wqk(h + 2, "k", nc.gpsimd)
                            if c == 2 and h % 2 == 1:
                                p = (h + 3) // 2
                                if p <= 3:
                                    dma_wv(p, nc.gpsimd)
                            for (dc_, which_, tr_) in qk_sched.get(s, []):
                                filler += qk_quarter(dc_, which_, tr_)
                            p = h // 2 + 1
                            if p <= 3:
                                tb = (h % 2) * 8 + c * 2
                                filler += v_group(p, tb)
                                filler += v_group(p, tb + 1)

                        fi = iter(filler)

                        def pull(n):
                            for _ in range(n):
                                t = next(fi, None)
                                if t is None:
                                    return
                                t()

                        # ---- S blocks + exp, interleaved with filler ----
                        if s < 32:
                            njb = 4 * (c + 1)
                            pt = ptpool.tile([128, 16, 512], BF16, tag="pt")
                            # clear the stale [0, lo) regions of the diagonal
                            # blocks (read by the staircase multiply / AV)
                            for sdg in range(1, 4):
                                nc.vector.memset(
                                    pt[:, 4 * c + sdg, 0:sdg * 128], 0.0)
                            for jb in range(njb):
                                sdg = jb - 4 * c
                                lo = sdg * 128 if sdg > 0 else 0
                                ps = psst.tile([128, 512], F32, tag="s",
                                               name="s")
                                nc.tensor.matmul(
                                    ps[:, lo:512],
                                    k_ring[h][:, jb * 128:(jb + 1) * 128],
                                    q_ring[h][:, c * 512 + lo:(c + 1) * 512],
                                    start=True,
                                    stop=True,
                                )
                                nc.scalar.activation(
                                    pt[:, jb, lo:512], ps[:, lo:512], Exp,
                                    bias=maskT_sb[:, jb:jb + 1],
                                )
                                pull(3)
                            # staircase zero of the diagonal blocks
                            nc.vector.tensor_tensor(
                                pt[:, 4 * c:4 * c + 4, :],
                                pt[:, 4 * c:4 * c + 4, :],
                                cdg01_sb[:, :, :], mult,
                            )
                            # pairwise partial-sum tree -> za, zb (bf16)
                            za = zpool.tile([128, 512], BF16, tag="za",
                                            name="za")
                            zb = zpool.tile([128, 512], BF16, tag="zb",
                                            name="zb")
                            nc.vector.tensor_tensor(za, pt[:, 0, :],
                                                    pt[:, 2, :], add)
                            nc.vector.tensor_tensor(zb, pt[:, 1, :],
                                                    pt[:, 3, :], add)
                            for base in range(4, njb, 2):
                                nc.vector.tensor_tensor(
                                    za, za, pt[:, base, :], add)
                                nc.vector.tensor_tensor(
                                    zb, zb, pt[:, base + 1, :], add)
                            state[(h, c)] = (pt, njb, za, zb)
                        # ---- flush remaining filler ----
                        pull(len(filler))

                # ---------- phase 3: out = yT^T @ WpT ----------
                with (
                    tc.tile_pool(name="wp", bufs=1) as wppool,
                    tc.tile_pool(name="p3ps", bufs=4, space="PSUM") as ps3,
                    tc.tile_pool(name="p3o", bufs=4) as op3,
                ):
                    wp_sb = wppool.tile([128, HG, C], BF16)
                    oq = [nc.sync, nc.gpsimd]
                    for hh in range(HG):
                        oq[hh % 2].dma_start(
                            out=wp_sb[:, hh, :],
                            in_=wpT[hh * 128:(hh + 1) * 128, :],
                        )
                    for tcb in range(16):
                        pss = [ps3.tile([128, 512], F32, tag="ps3",
                                        name=f"ps3_{cr}")
                               for cr in range(4)]
                        for hh in range(HG):
                            for cr in range(4):
                                nc.tensor.matmul(
                                    pss[cr],
                                    yT_sb[:, hh, tcb * 128:(tcb + 1) * 128],
                                    wp_sb[:, hh, cr * 512:(cr + 1) * 512],
                                    start=(hh == 0),
                                    stop=(hh == HG - 1),
                                )
                        for cr in range(4):
                            ob = op3.tile([128, 512], BF16, tag="ob")
                            nc.scalar.activation(ob, pss[cr], Copy)
                            (nc.sync if cr % 2 == 0 else nc.gpsimd).dma_start(
                                out=out[tcb * 128:(tcb + 1) * 128,
                                        cr * 512:(cr + 1) * 512],
                                in_=ob,
                            )
    nc.compile()
    return nc


def get_nc():
    global _NC_CACHE
    if _NC_CACHE is None:
        _NC_CACHE = _build_program()
    return _NC_CACHE


def prep_core_inputs(inputs):
    """Host-side sharding / layout prep: slice per (b, g), transpose to the
    layouts the device program wants, fold the 1/sqrt(d) softmax scale into
    Wq/bq."""
    f = lambda a: np.asarray(a, dtype=np.float32)
    bf = ml_dtypes.bfloat16
    x = f(inputs["x"])
    am = f(inputs["attn_mask"])
    Wq, bq_ = f(inputs["Wq"]), f(inputs["bq"])
    Wk, bk_ = f(inputs["Wk"]), f(inputs["bk"])
    Wv, bv_ = f(inputs["Wv"]), f(inputs["bv"])
    Wp = f(inputs["Wp"])
    scale = 1.0 / math.sqrt(D)

    # 0/1 staircase in S^T layout: for diagonal block s (0..3) of a 512-wide
    # query chunk, partition p = key offset within the 128-block, column
    # i_local in [0, 512): masked (dead) iff i_local < s*128 + p.
    ii = np.arange(512)[None, :]
    pp = np.arange(128)[:, None]
    cdg01_t = np.stack(
        [np.where(ii < s * 128 + pp, 0.0, 1.0) for s in range(4)], axis=1
    ).astype(bf)  # [128, 4, 512]

    per_g = []
    for g in range(2):
        sl = slice(g * CG, (g + 1) * CG)
        per_g.append(dict(
            wqT=(np.ascontiguousarray(Wq[sl].T) * scale).astype(bf),
            wkT=np.ascontiguousarray(Wk[sl].T).astype(bf),
            wvT=np.ascontiguousarray(Wv[sl].T).astype(bf),
            bq=np.ascontiguousarray((bq_[sl] * scale).reshape(HG, 128).T),
            bk=np.ascontiguousarray(bk_[sl].reshape(HG, 128).T),
            bvb=np.ascontiguousarray(
                np.broadcast_to(bv_[sl], (128, CG))
            ).astype(bf),
            wpT=np.ascontiguousarray(Wp[:, sl].T).astype(bf),
        ))

    ones_t = np.ones((128, 128), dtype=bf)

    in_maps = []
    for core in range(N_CORES):
        b, g = core // 2, core % 2
        m = dict(per_g[g])
        m["xT"] = np.ascontiguousarray(x[b].T).astype(bf)
        m["maskT"] = np.ascontiguousarray(
            am[b, 0, 0, :].reshape(16, 128).T
        )
        m["cdg01"] = cdg01_t
        m["ones128"] = ones_t
        in_maps.append(m)
    return in_maps


def run(inputs, trace=False):
    nc = get_nc()
    in_maps = prep_core_inputs(inputs)
    rr = run_bass_kernel_spmd(nc, in_maps, list(range(N_CORES)), trace=trace)
    bp = np.asarray(inputs["bp"], dtype=np.float32)
    y = np.empty((B, T, C), dtype=np.float32)
    for b in range(B):
        y[b] = (np.asarray(rr.results[2 * b]["out"], dtype=np.float32)
                + np.asarray(rr.results[2 * b + 1]["out"], dtype=np.float32)
                + bp[None, :])
    return y, rr


def kernel(**inputs):
    y, _ = run(inputs)
    return y
